# revision 18
# baseline (speedup 1.0000x reference)
"""Trainium2 Bass kernel for nn_Attention_53798760350139.

Module: x + pos_enc -> unscaled self-attention (softmax(x x^T) x) -> MLP ->
residual -> full-sample layernorm.  B=16, H=W=48, D=384.

Sharding: data-parallel over batch across 8 cores (2 batches per core),
weights replicated.  Inputs are FULL tensors; output is the FULL tensor.

Attention strategy: scores are symmetric (S = X X^T), so only the upper
triangle of 128x128 token blocks is computed (super-row r computes blocks
(r, c>=r); each block serves row r via an XBAR DMA transpose and row c
directly).  x is centered on the host (y = x - mean_token); softmax is
shift-invariant per row, so only the u_j = c . y_j terms matter.  Both the
per-partition (u_a) and per-column (u_b) terms are folded into the score
matmul itself via one extra 4-partition f16 matmul per chunk (hi/lo-split
f16 carries u exactly to ~0.03), so stored scores are yy + u_a + u_b and
the row max (computed per PSUM chunk on DVE, combined) is the exact
stabilizer.  The direct path is then a plain per-column subtract (Pool)
followed by one WIDE bias-free exp (ACT) per row; the transposed path is a
wide exp-first (bias -max per partition) into f16, transposed SBUF->SBUF by
the DMA XBAR (no PE), then converted f16->f8 by DVE/Pool.  Unnormalized f8
probabilities feed a DoubleRow fp8 AV matmul with a ones-augmented x
operand so the row sum l rides along; normalization happens on the AV
output.  The MLP runs in f16 with pair-batched MLP1 (256-wide moving).
Rows are processed in software-pipelined pairs; each batch's layernorm
finalize is deferred past the next batch's start.
"""
import numpy as np
import ml_dtypes
from contextlib import ExitStack

import concourse.bass as bass
import concourse.tile as tile
from concourse import bacc, mybir
from concourse.bass_utils import run_bass_kernel_spmd
from concourse.masks import make_identity
from concourse.bass import ts

F32 = mybir.dt.float32
F16 = mybir.dt.float16
F8 = mybir.dt.float8e4
BF16 = mybir.dt.bfloat16
AX = mybir.AxisListType
OP = mybir.AluOpType
AF = mybir.ActivationFunctionType

B, H, W, D = 16, 48, 48, 384
NT = H * W          # 2304 tokens
NCORES = 8
BPC = B // NCORES   # 2 batches per core
KT = D // 128       # 3 contraction tiles over D
TB = NT // 128      # 18 token blocks
NTRI = TB * (TB + 1) // 2   # 171 upper-triangle blocks
EPS = 1e-5
NEG = -3.0e38

_prog_cache = {}


def _tri(r, c):
    """r-major upper-triangle block index for r <= c."""
    return r * TB - r * (r - 1) // 2 + (c - r)


def _build_program():
    nc = bacc.Bacc("TRN2", target_bir_lowering=False, debug=False)

    yt_d = nc.dram_tensor("yt", [BPC, 128, KT, NT], F16, kind="ExternalInput").ap()
    w_d = nc.dram_tensor("wrow", [BPC, 1, D], F16, kind="ExternalInput").ap()
    ua_d = nc.dram_tensor("ua6", [BPC, 6, NT], F16, kind="ExternalInput").ap()
    ub_d = nc.dram_tensor("ub6", [BPC, 6, NT], F16, kind="ExternalInput").ap()
    w1_d = nc.dram_tensor("w1", [128, KT, D], F16, kind="ExternalInput").ap()
    w2_d = nc.dram_tensor("w2", [128, KT, D], F16, kind="ExternalInput").ap()
    b1_d = nc.dram_tensor("b1c", [128, KT, 1], F32, kind="ExternalInput").ap()
    out_d = nc.dram_tensor("out", [BPC, 128, TB, D], F16, kind="ExternalOutput").ap()

    with tile.TileContext(nc) as tc, ExitStack() as ctx:
        const = ctx.enter_context(tc.tile_pool(name="const", bufs=1))
        inp = ctx.enter_context(tc.tile_pool(name="inp", bufs=1))
        yt_pool = ctx.enter_context(tc.tile_pool(name="ytp", bufs=2))
        store_p = ctx.enter_context(tc.tile_pool(name="store", bufs=1))
        acc_p = ctx.enter_context(tc.tile_pool(name="acc", bufs=1))
        pt_pool = ctx.enter_context(tc.tile_pool(name="pt", bufs=3))
        scr_p = ctx.enter_context(tc.tile_pool(name="scr", bufs=1))
        f16s_p = ctx.enter_context(tc.tile_pool(name="f16s", bufs=2))
        f16t_p = ctx.enter_context(tc.tile_pool(name="f16t", bufs=1))
        sml = ctx.enter_context(tc.tile_pool(name="sml", bufs=3))
        sml2 = ctx.enter_context(tc.tile_pool(name="sml2", bufs=2))
        ot_p = ctx.enter_context(tc.tile_pool(name="otp", bufs=1))
        sq_p = ctx.enter_context(tc.tile_pool(name="sq", bufs=1))
        ln_p = ctx.enter_context(tc.tile_pool(name="ln", bufs=2))
        ps_sc = ctx.enter_context(tc.tile_pool(name="ps_sc", bufs=2, space="PSUM"))
        ps_oa = ctx.enter_context(tc.tile_pool(name="ps_oa", bufs=2, space="PSUM"))
        ps_h = ctx.enter_context(tc.tile_pool(name="ps_h", bufs=1, space="PSUM"))
        ps_m = ctx.enter_context(tc.tile_pool(name="ps_m", bufs=1, space="PSUM"))

        # ---------- constants / weights ----------
        identf = const.tile([128, 128], F32, tag="identf")
        make_identity(nc, identf[:])
        ones_col = const.tile([128, 1], F32, tag="ones_col")
        nc.vector.memset(ones_col[:], 1.0)
        ones_row = const.tile([1, 128], F32, tag="ones_row")
        nc.vector.memset(ones_row[:], 1.0)
        ones_row16 = const.tile([1, 128], F16, tag="ones_row16")
        nc.vector.memset(ones_row16[:], 1.0)

        w1b = const.tile([128, KT, D], F16, tag="w1b")
        w2b = const.tile([128, KT, D], F16, tag="w2b")
        nc.sync.dma_start(w1b[:], w1_d)
        nc.sync.dma_start(w2b[:], w2_d)
        b1_t = const.tile([128, KT, 1], F32, tag="b1t")
        nc.sync.dma_start(b1_t[:], b1_d)

        def emit_ln(bb, stats, xn):
            # layernorm finalize + store (deferred past next batch start)
            pstat = ps_oa.tile([128, 512], F32, tag="oa")
            nc.tensor.matmul(
                pstat[:1, : 2 * TB],
                ones_col[:],
                stats[:].rearrange("p a b -> p (a b)"),
                start=True,
                stop=True,
            )
            tot = ln_p.tile([1, 2], F32, tag="tot")
            nc.vector.tensor_reduce(
                tot[:],
                pstat[:1, : 2 * TB].rearrange("p (a b) -> p a b", a=2),
                axis=AX.X,
                op=OP.add,
            )
            NALL = float(NT * D)
            mv = ln_p.tile([1, 2], F32, tag="mv")  # [mean, e2]
            nc.vector.tensor_scalar_mul(mv[:], tot[:], 1.0 / NALL)
            msq = ln_p.tile([1, 1], F32, tag="msq")
            nc.vector.tensor_tensor(msq[:], mv[:, :1], mv[:, :1], OP.mult)
            vare = ln_p.tile([1, 1], F32, tag="vare")
            nc.vector.tensor_tensor(vare[:], mv[:, 1:2], msq[:], OP.subtract)
            nc.vector.tensor_scalar_add(vare[:], vare[:], EPS)
            sd = ln_p.tile([1, 1], F32, tag="sd")
            nc.scalar.sqrt(sd[:], vare[:])
            r0 = ln_p.tile([1, 1], F32, tag="r0")
            nc.vector.reciprocal(r0[:], sd[:])
            # one Newton step for rsqrt accuracy
            t_a = ln_p.tile([1, 1], F32, tag="ta")
            nc.vector.tensor_tensor(t_a[:], r0[:], r0[:], OP.mult)
            nc.vector.tensor_tensor(t_a[:], t_a[:], vare[:], OP.mult)
            nc.vector.tensor_scalar(
                t_a[:], t_a[:], -0.5, 1.5, OP.mult, OP.add
            )
            r1 = ln_p.tile([1, 1], F32, tag="r1")
            nc.vector.tensor_tensor(r1[:], r0[:], t_a[:], OP.mult)
            mr = ln_p.tile([1, 2], F32, tag="mr")
            nc.vector.tensor_copy(mr[:, :1], mv[:, :1])
            nc.vector.tensor_copy(mr[:, 1:2], r1[:])
            pbc = ps_m.tile([128, 128], F32, tag="mh")
            nc.tensor.matmul(pbc[:, :2], ones_row[:], mr[:], start=True, stop=True)
            mrb = ln_p.tile([128, 2], F32, tag="mrb")
            nc.vector.tensor_copy(mrb[:], pbc[:, :2])
            last = bb == BPC - 1
            nb1 = ln_p.tile([128, 1], F32, tag="nb1")
            nc.vector.scalar_tensor_tensor(
                nb1[:], mrb[:, 0:1], -1.0, mrb[:, 1:2], OP.mult, OP.mult
            )
            for ib in range(TB):
                if last and ib % 3 == 1:
                    nc.scalar.activation(
                        xn[:, ib, :], xn[:, ib, :], AF.Identity,
                        bias=nb1[:], scale=mrb[:, 1:2],
                    )
                elif last and ib % 3 == 2:
                    nc.gpsimd.tensor_scalar(
                        xn[:, ib, :], xn[:, ib, :],
                        mrb[:, 0:1], mrb[:, 1:2],
                        OP.subtract, OP.mult,
                    )
                else:
                    nc.vector.tensor_scalar(
                        xn[:, ib, :], xn[:, ib, :],
                        mrb[:, 0:1], mrb[:, 1:2],
                        OP.subtract, OP.mult,
                    )
                if last and ib % 3 == 2:
                    nc.sync.dma_start(
                        out_d[bb, :, ib - 2 : ib + 1], xn[:, ib - 2 : ib + 1, :]
                    )
            if not last:
                nc.sync.dma_start(out_d[bb], xn[:])

        _pending_ln = []

        _next_yt = {}

        for b in range(BPC):
            # ---------- load batch ----------
            if b in _next_yt:
                yt = _next_yt.pop(b)
            else:
                yt = yt_pool.tile([128, KT, NT], F16, tag="yt")
                for k in range(KT):
                    nc.sync.dma_start(yt[:, k, :], yt_d[b, :, k, :])
            ua6 = inp.tile([6, NT], F16, tag="ua6")
            nc.sync.dma_start(ua6[:], ua_d[b])
            ub6 = inp.tile([6, NT], F16, tag="ub6")
            nc.sync.dma_start(ub6[:], ub_d[b])
            wrow = inp.tile([1, D], F16, tag="wrow")
            nc.sync.dma_start(wrow[:], w_d[b])
            # xn = transpose(yt) + (c + b2) broadcast; xa = f8(xn) | ones col
            xn = inp.tile([128, TB, D], F16, tag="xn")
            for k in range(KT):
                nc.sync.dma_start_transpose(xn[:, :, ts(k, 128)], yt[:, k, :])
            wb_ps = ps_m.tile([128, 512], F32, tag="wbp")
            nc.tensor.matmul(
                wb_ps[:, :D], ones_row16[:], wrow[:], start=True, stop=True
            )
            wb = inp.tile([128, 1, D], F16, tag="wb")
            nc.vector.tensor_copy(wb[:, 0, :], wb_ps[:, :D])
            nc.vector.tensor_tensor(
                xn[:], xn[:], wb[:].broadcast_to([128, TB, D]), OP.add
            )
            xa = yt_pool.tile([128, TB, D + 1], F8, tag="xa")
            nc.vector.memset(xa[:, :, D : D + 1], 1.0)
            nc.gpsimd.tensor_copy(xa[:, :4, :D], xn[:, :4, :])
            nc.vector.tensor_copy(xa[:, 4:11, :D], xn[:, 4:11, :])
            nc.scalar.copy(xa[:, 11:, :D], xn[:, 11:, :])

            stb = store_p.tile([128, NTRI, 128], F32, tag="stb")
            stats = acc_p.tile([128, 2, TB], F32, tag="stats")

            # per-row state carried across pipeline stages
            mrow = {}    # r -> [128,1] exact row max (incl. u_a + u_b fold)
            negmb = {}   # P -> [128,2,128] broadcast of -(m+u) per pair column
            pt_bufs = {}
            ot_bufs = {}

            def emit_scores(r):
                """Upper-triangle score blocks for super-row r: yy + u_a + u_b
                accumulated in PSUM (u via one 4-partition f16 matmul), stored
                by ACT, per-chunk row max on DVE."""
                coff = r
                pmaxp = sml.tile([128, 8], F32, tag="pmaxp")
                ci = 0
                while coff < TB:
                    ncb = min(4, TB - coff)
                    w = ncb * 128
                    psc = ps_sc.tile([128, 512], F32, tag="psc")
                    nc.tensor.matmul(
                        psc[:, :w],
                        ua6[:, ts(r, 128)],
                        ub6[:, coff * 128 : coff * 128 + w],
                        start=True,
                        stop=False,
                    )
                    for k in range(KT):
                        nc.tensor.matmul(
                            psc[:, :w],
                            yt[:, k, ts(r, 128)],
                            yt[:, k, coff * 128 : coff * 128 + w],
                            start=False,
                            stop=(k == KT - 1),
                        )
                    t0 = _tri(r, coff)
                    nc.scalar.copy(
                        stb[:, t0 : t0 + ncb, :].rearrange("p a b -> p (a b)"),
                        psc[:, :w],
                    )
                    nc.vector.tensor_reduce(
                        pmaxp[:, ci : ci + 1], psc[:, :w], axis=AX.X, op=OP.max
                    )
                    ci += 1
                    coff += ncb
                pmax = sml.tile([128, 1], F32, tag="pmax")
                nc.vector.tensor_reduce(
                    pmax[:], pmaxp[:, :ci], axis=AX.X, op=OP.max
                )
                mrow[r] = pmax
                return pmax

            def emit_mfin_te(P, rsub, v_t):
                """Broadcast m (=pmax) for row 2P+rsub into negmb[P][:, rsub, :]."""
                if rsub == 0:
                    nmb = sml2.tile([128, 2, 128], F32, tag="nmb")
                    negmb[P] = nmb
                nmb = negmb[P]
                tpv = ps_m.tile([128, 256], F32, tag="mh")
                nc.tensor.transpose(tpv[:1, :128], v_t[:], identf[:])
                vrow = inp.tile([1, 128], F32, tag="vrow")
                nc.vector.tensor_copy(vrow[:], tpv[:1, :128])
                psb = ps_m.tile([128, 256], F32, tag="mh")
                nc.tensor.matmul(
                    psb[:, :128], ones_row[:], vrow[:], start=True, stop=True
                )
                nc.vector.tensor_copy(nmb[:, rsub, :], psb[:, :128])

            def emit_ptA(P):
                """Direct-path operand blocks for pair P: per-j Pool subtract
                into scr, then one wide bias-free ACT exp per sub-row."""
                r0, r1 = 2 * P, 2 * P + 1
                ptb = pt_pool.tile([128, 2, TB, 128], F8, tag="pt")
                pt_bufs[P] = ptb
                nmb = negmb.pop(P)
                scr = scr_p.tile([128, TB, 2, 128], F16, tag="scr")
                for j in range(r0 + 1):
                    t = _tri(j, r0)
                    eng = nc.gpsimd if j % 2 == 0 else nc.vector
                    eng.tensor_tensor(
                        scr[:, j, :, :],
                        stb[:, t : t + 2, :],
                        nmb[:],
                        OP.subtract,
                    )
                # odd-row diagonal block (r1, r1): single
                t = _tri(r1, r1)
                nc.gpsimd.tensor_tensor(
                    scr[:, r1, 1, :], stb[:, t, :], nmb[:, 1, :], OP.subtract
                )
                nc.scalar.activation(
                    ptb[:, 0, : r0 + 1, :], scr[:, : r0 + 1, 0, :], AF.Exp
                )
                nc.scalar.activation(
                    ptb[:, 1, : r1 + 1, :], scr[:, : r1 + 1, 1, :], AF.Exp
                )
                # exp-first f16 for the transposed parts (wide, bias -m)
                f16d = {}
                for rr in (r0, r1):
                    nt_ = TB - 1 - rr
                    if nt_ == 0:
                        continue
                    m_t = mrow.pop(rr)
                    mneg = sml.tile([128, 1], F32, tag="mneg")
                    nc.vector.tensor_scalar_mul(mneg[:], m_t[:], -1.0)
                    f16s = f16s_p.tile([128, TB - 1, 128], F16, tag="f16s")
                    f16d[rr] = f16s
                    t0 = _tri(rr, rr + 1)
                    nc.scalar.activation(
                        f16s[:, :nt_, :].rearrange("p a b -> p (a b)"),
                        stb[:, t0 : t0 + nt_, :].rearrange("p a b -> p (a b)"),
                        AF.Exp, bias=mneg[:], scale=1.0,
                    )
                return f16d

            def emit_ptB(P, f16d):
                """Transposed-path operands for pair P (XBAR DMA transposes +
                f16->f8 converts on DVE/Pool)."""
                r0 = 2 * P
                ptb = pt_bufs[P]
                for rr, rsub in ((r0, 0), (r0 + 1, 1)):
                    nt_ = TB - 1 - rr
                    if nt_ == 0:
                        continue
                    f16s = f16d[rr]
                    f16sT = f16t_p.tile([128, TB - 1, 128], F16, tag="f16sT")
                    nc.sync.dma_start_transpose(
                        f16sT[:, :nt_, :],
                        f16s[:, :nt_, :].rearrange("p a b -> p (a b)"),
                    )
                    dst = ptb[:, rsub, rr + 1 : TB, :]
                    src = f16sT[:, :nt_, :]
                    if rr < 10:
                        nc.vector.tensor_copy(dst, src)
                    else:
                        nc.gpsimd.tensor_copy(dst, src)

            def emit_tail_av(r):
                """AV + normalize + XBAR transpose of the normalized output."""
                ptb = pt_bufs[r // 2]
                rsub = r % 2
                oa = ps_oa.tile([128, 512], F32, tag="oa")
                NJP = TB // 2
                for jp in range(NJP):
                    nc.tensor.matmul(
                        oa[:, : D + 1],
                        ptb[:, rsub, 2 * jp : 2 * jp + 2, :],
                        xa[:, 2 * jp : 2 * jp + 2, :],
                        start=(jp == 0),
                        stop=(jp == NJP - 1),
                        perf_mode=mybir.MatmulPerfMode.DoubleRow,
                    )
                rl = sml.tile([128, 1], F32, tag="rl")
                nc.vector.reciprocal(rl[:], oa[:, D : D + 1])
                P = r // 2
                if rsub == 0:
                    obf2 = sml2.tile([128, 2, D], F16, tag="obf2")
                    ot_bufs[P] = (obf2, None)
                obf2 = ot_bufs[P][0]
                if rsub == 0:
                    nc.scalar.activation(
                        obf2[:, 0, :], oa[:, :D], AF.Copy, scale=rl[:]
                    )
                else:
                    nc.vector.tensor_scalar_mul(obf2[:, 1, :], oa[:, :D], rl[:])
                    oT2 = ot_p.tile([128, 2, KT, 128], F16, tag="oT2")
                    ot_bufs[P] = (obf2, oT2)
                    nc.sync.dma_start_transpose(
                        oT2[:].rearrange("p a k c -> p (a k) c"),
                        obf2[:].rearrange("p a b -> p (a b)"),
                    )

            def emit_tail_mlp(P):
                """Pair-batched MLP1 + relu + per-row MLP2 + residual + stats."""
                oT2 = ot_bufs.pop(P)[1]
                r0 = 2 * P
                # MLP1: hT2[m, (rsub, c)] = sum_k w1[k, m]^T oT2[k, rsub, c]
                hps = ps_h.tile([128, KT, 2, 128], F32, tag="hps")
                for m in range(KT):
                    for k in range(KT):
                        nc.tensor.matmul(
                            hps[:, m, :, :],
                            w1b[:, k, ts(m, 128)],
                            oT2[:, :, k, :],
                            start=(k == 0),
                            stop=(k == KT - 1),
                        )
                hT2 = sml2.tile([128, KT, 2, 128], F16, tag="hT2")
                for m in range(KT):
                    if m == 1:
                        nc.vector.tensor_scalar(
                            hT2[:, m, :, :], hps[:, m, :, :],
                            b1_t[:, m, :], 0.0, OP.add, OP.max,
                        )
                    else:
                        nc.scalar.activation(
                            hT2[:, m, :, :], hps[:, m, :, :], AF.Relu,
                            bias=b1_t[:, m, :], scale=1.0,
                        )
                # MLP2 + residual (+ b2 folded into xn on host) + stats
                mps = ps_h.tile([128, 2, 512], F32, tag="hps")
                for rsub in range(2):
                    r = r0 + rsub
                    for m in range(KT):
                        nc.tensor.matmul(
                            mps[:, rsub, :D], hT2[:, m, rsub, :], w2b[:, m, :],
                            start=(m == 0), stop=(m == KT - 1),
                        )
                    nc.vector.scalar_tensor_tensor(
                        xn[:, r, :], mps[:, rsub, :D], 1.0, xn[:, r, :],
                        OP.mult, OP.add,
                        accum_out=stats[:, 0, r : r + 1],
                    )
                    sqd = sq_p.tile([128, D], F16, tag="sq")
                    nc.scalar.activation(
                        sqd[:], xn[:, r, :], AF.Square,
                        accum_out=stats[:, 1, r : r + 1],
                    )

            # ---------- software-pipelined super-row-pair loop ----------
            NP = TB // 2
            f16d = None
            for P in range(NP):
                r0, r1 = 2 * P, 2 * P + 1
                if P > 0:
                    f16d = emit_ptA(P - 1)
                pm0 = emit_scores(r0)
                if P > 0:
                    emit_ptB(P - 1, f16d)
                if P > 1:
                    emit_tail_av(2 * (P - 2))
                emit_mfin_te(P, 0, pm0)
                pm1 = emit_scores(r1)
                if P > 1:
                    emit_tail_av(2 * (P - 2) + 1)
                    emit_tail_mlp(P - 2)
                emit_mfin_te(P, 1, pm1)
                if P == 0 and _pending_ln:
                    emit_ln(*_pending_ln.pop(0))
                if P == 5 and b + 1 < BPC:
                    nyt = yt_pool.tile([128, KT, NT], F16, tag="yt")
                    for k in range(KT):
                        nc.sync.dma_start(nyt[:, k, :], yt_d[b + 1, :, k, :])
                    _next_yt[b + 1] = nyt
            f16d = emit_ptA(NP - 1)
            emit_ptB(NP - 1, f16d)
            emit_tail_av(2 * (NP - 2))
            emit_tail_av(2 * (NP - 2) + 1)
            emit_tail_mlp(NP - 2)
            emit_tail_av(TB - 2)
            emit_tail_av(TB - 1)
            emit_tail_mlp(NP - 1)

            _pending_ln.append((b, stats, xn))

        for args in _pending_ln:
            emit_ln(*args)

    nc.compile()
    return nc


def _host_prep(x, Wp, bp, b2):
    ph = np.arange(H, dtype=np.float32)[:, None] * np.ones((1, W), np.float32)
    pw = np.arange(W, dtype=np.float32)[None, :] * np.ones((H, 1), np.float32)
    pos = np.stack((ph, pw), axis=-1).reshape(NT, 2)
    pos_enc = pos @ Wp.astype(np.float32) + bp.astype(np.float32)
    xf = x.reshape(B, NT, D).astype(np.float32) + pos_enc[None]
    c = xf.mean(axis=1, keepdims=True)                    # (B,1,D)
    y = xf - c
    u = np.einsum(
        "bod,bnd->bn", c.astype(np.float64), y.astype(np.float64)
    ).astype(np.float32)                                  # (B,NT)
    yq = y.astype(np.float16)
    # yt[b, p, k, t] = y[b, t, k*128+p]
    yt = np.ascontiguousarray(yq.reshape(B, NT, KT, 128).transpose(0, 3, 2, 1))
    wv = (c[:, 0, :] + b2.astype(np.float32)[None]).astype(np.float16)
    wv = wv[:, None, :]  # (B, 1, D): c + b2, re-added on device
    # 3-term f16 split of u/2 (|u| can exceed f16 max) for the exact
    # in-matmul fold of u_a + u_b: out += sum_i lhs_i[a]*rhs_i[c]
    uh = (u * 0.5).astype(np.float64)
    u1 = uh.astype(np.float16)
    u2 = (uh - u1.astype(np.float64)).astype(np.float16)
    u3 = (uh - u1.astype(np.float64) - u2.astype(np.float64)).astype(np.float16)
    twos = np.full_like(u1, 2.0)
    ua6 = np.ascontiguousarray(
        np.stack([u1, u2, u3, twos, twos, twos], axis=1)
    )  # (B, 6, NT) lhsT rows
    ub6 = np.ascontiguousarray(
        np.stack([twos, twos, twos, u1, u2, u3], axis=1)
    )  # (B, 6, NT) rhs rows
    return yt, wv, ua6, ub6


def _make_in_maps(inputs):
    x, Wp, bp = inputs["x"], inputs["Wp"], inputs["bp"]
    W1, b1, W2, b2 = inputs["W1"], inputs["b1"], inputs["W2"], inputs["b2"]
    yt, wv, ua6, ub6 = _host_prep(
        np.asarray(x, np.float32), np.asarray(Wp, np.float32),
        np.asarray(bp, np.float32), np.asarray(b2, np.float32),
    )
    w1t = np.ascontiguousarray(
        np.asarray(W1, np.float16).reshape(KT, 128, D).transpose(1, 0, 2)
    )
    w2t = np.ascontiguousarray(
        np.asarray(W2, np.float16).reshape(KT, 128, D).transpose(1, 0, 2)
    )
    b1p = (np.asarray(b1, np.float64)
           - np.asarray(b2, np.float64) @ np.asarray(W1, np.float64)
           ).astype(np.float32)
    b1c = np.ascontiguousarray(b1p.reshape(KT, 128).T[:, :, None])
    in_maps = []
    for core in range(NCORES):
        s = slice(core * BPC, (core + 1) * BPC)
        in_maps.append({
            "yt": np.ascontiguousarray(yt[s]),
            "wrow": np.ascontiguousarray(wv[s]),
            "ua6": np.ascontiguousarray(ua6[s]),
            "ub6": np.ascontiguousarray(ub6[s]),
            "w1": w1t,
            "w2": w2t,
            "b1c": b1c,
        })
    return in_maps


def kernel(x, Wp, bp, W1, b1, W2, b2):
    inputs = {
        "x": x, "Wp": Wp, "bp": bp, "W1": W1, "b1": b1, "W2": W2, "b2": b2,
    }
    in_maps = _make_in_maps(inputs)

    if "nc" not in _prog_cache:
        _prog_cache["nc"] = _build_program()
    nc = _prog_cache["nc"]

    res = run_bass_kernel_spmd(nc, in_maps, core_ids=list(range(NCORES)))
    _prog_cache["last_results"] = res
    out = np.concatenate([r["out"] for r in res.results], axis=0)
    # out[b, p, tb, d] -> [b, tb*128+p, d]
    out = out.transpose(0, 2, 1, 3).reshape(B, NT, D)
    return out.reshape(B, H, W, D).astype(np.float32)
